# revision 46
# baseline (speedup 1.0000x reference)
"""Multi-head attention kernel for Trainium2, SPMD over 8 NeuronCores.

Problem: B=4, N=2048, C=1024, 16 heads, head_dim=64 (fp32 reference).

Sharding: core = (batch b, head-group hg) with b = core//2, hg = core%2.
Each core computes attention for its 8 heads of its batch and a PARTIAL
projection output [N, C]; the host sums the two partials per batch and adds
the bias. No on-chip collectives needed.

Per-core dataflow (all matmul inputs bf16, f32 PSUM accumulation):
  qkT[o,n]  = w_qk_local @ x^T          (o = [q heads | k heads], 1024 rows)
  v[n,dl]   = x @ w_v_local^T           (natural layout, augmented with ones col)
  S_T[k,q]  = kT_h^T q (per head)       -> exp(0.125*S_T) on ScalarE -> E bf16
  numT[d,q] = v_aug^T E  (M=65: row 64 = sumexp)   accumulated over k chunks
  outT[d,q] = numT[0:64] * (1/sumexp)   (recip via DMA gather + DVE, DMA bcast)
  partial[n,o] = outT^T @ w_proj_local^T chunks    -> DMA out (f32)
"""
import os
import sys
import types
import time
import numpy as np
import ml_dtypes
from contextlib import ExitStack

import concourse.bass as bass
import concourse.tile as tile
from concourse import bacc, library_config, mybir

BF16 = mybir.dt.bfloat16
F32 = mybir.dt.float32

N = 2048          # sequence length
C = 1024          # model dim
HL = 8            # heads per core
D = 64            # head dim
SCALE = D ** -0.5
NCORES = 8

KC = N // 128     # 16 k-chunks per head
QB = 4            # q blocks of 512
QBS = N // QB     # 512


# ---------------------------------------------------------------------------
# axon NTFF-profiling hook shim (the container's antenv lacks axon_hooks)
# ---------------------------------------------------------------------------
def _install_ntff_hook():
    if "antenv.axon_hooks" in sys.modules:
        return
    mod = types.ModuleType("antenv.axon_hooks")
    _state = {"hook": None}
    mod.set_axon_ntff_profile_hook = lambda h: _state.__setitem__("hook", h)
    mod.get_axon_ntff_profile_hook = lambda: _state["hook"]
    sys.modules["antenv.axon_hooks"] = mod
    try:
        import antenv
        antenv.axon_hooks = mod
    except ImportError:
        pass
    try:
        if "/root/.axon_site" not in sys.path:
            sys.path.insert(0, "/root/.axon_site")
        from trn_agent_boot.trn_boot import _ntff_profile_via_ctypes
        mod.set_axon_ntff_profile_hook(
            _ntff_profile_via_ctypes("/opt/axon/libaxon_pjrt.so")
        )
    except Exception:
        pass
    try:
        import concourse.bass_utils as bu
        bu.upload_artifacts = lambda tmpdir: tmpdir
    except Exception:
        pass


# ---------------------------------------------------------------------------
# kernel build
# ---------------------------------------------------------------------------
_CACHE = {}


def _build():
    if "nc" in _CACHE:
        return _CACHE["nc"]
    nc = bacc.Bacc("TRN2", target_bir_lowering=False, debug=False,
                   num_devices=NCORES)

    xt_d = nc.dram_tensor("x_t", [C, N], BF16, kind="ExternalInput").ap()
    wqkt_d = nc.dram_tensor("w_qkt", [C, 2 * HL * D], BF16,
                            kind="ExternalInput").ap()
    wvt_d = nc.dram_tensor("w_vt", [C, HL * D], BF16,
                           kind="ExternalInput").ap()
    wprojt_d = nc.dram_tensor("w_projt", [HL * D, C], BF16,
                              kind="ExternalInput").ap()
    out_d = nc.dram_tensor("out", [N, C], F32, kind="ExternalOutput").ap()

    dumps = bool(int(os.environ.get("KERNEL_DUMPS", "0")))
    if dumps:
        qk_dump = nc.dram_tensor("qk_dump", [128, C // 128, N], BF16,
                                 kind="ExternalOutput").ap()
        vaug_dump = nc.dram_tensor("vaug_dump", [128, KC, HL, 72], BF16,
                                   kind="ExternalOutput").ap()
        outT_dump = nc.dram_tensor("outT_dump", [128, HL * D // 128, N], BF16,
                                   kind="ExternalOutput").ap()

    CCH = C // 128    # 8 contraction chunks

    with tile.TileContext(nc) as tc:
        with ExitStack() as ctx:
            pers = ctx.enter_context(tc.tile_pool(name="pers", bufs=1))
            e_pool = ctx.enter_context(tc.tile_pool(name="e", bufs=8))
            tiny = ctx.enter_context(tc.tile_pool(name="tiny", bufs=4))
            stage = ctx.enter_context(tc.tile_pool(name="stage", bufs=3))
            ppool = ctx.enter_context(tc.tile_pool(name="ppool", bufs=8))
            ps_sc = ctx.enter_context(
                tc.tile_pool(name="ps_sc", bufs=2, space="PSUM"))
            ps_out = ctx.enter_context(
                tc.tile_pool(name="ps_out", bufs=1, space="PSUM"))
            ps_mm = ctx.enter_context(
                tc.tile_pool(name="ps_mm", bufs=2, space="PSUM"))


            # persistent SBUF tensors
            xt = pers.tile([128, CCH, N], BF16)          # x^T   (c,n)
            wqkt = pers.tile([128, CCH, 2 * HL * D], BF16)
            wvt = pers.tile([128, CCH, HL * D], BF16)
            wprojt = pers.tile([128, HL * D // 128, C], BF16)
            qk = pers.tile([128, CCH, N], BF16)          # qkT rows (o,n)
            # innermost padded 65->72 so each [*, kc, h, 0:65] LDWEIGHTS
            # slice starts 16B-aligned (misaligned weight reads cost ~30ns)
            v_aug = pers.tile([128, KC, HL, 72], BF16)
            outT = pers.tile([128, HL * D // 128, N], BF16)

            # input DMAs, ordered by the first exp's critical path: x seq
            # 0:512 + wqkt gate the first kT/qT tiles (and the exp chain
            # start); wvt/A tiles aren't needed until the first AV, 3 steps
            # later. Everything else follows in consumption order.
            nc.gpsimd.load_library(library_config.attn)
            for cc in range(CCH):
                nc.sync.dma_start(xt[:, cc, 0:512],
                                  xt_d[cc * 128:(cc + 1) * 128, 0:512])
            for cc in range(CCH):
                nc.sync.dma_start(wqkt[:, cc, :], wqkt_d[cc * 128:(cc + 1) * 128, :])
            for cc in range(CCH):
                nc.sync.dma_start(wvt[:, cc, :], wvt_d[cc * 128:(cc + 1) * 128, :])
            for cc in range(CCH):
                nc.sync.dma_start(xt[:, cc, 512:1024],
                                  xt_d[cc * 128:(cc + 1) * 128, 512:1024])
            for cc in range(CCH):
                nc.sync.dma_start(xt[:, cc, 1024:1536],
                                  xt_d[cc * 128:(cc + 1) * 128, 1024:1536])
            for cc in range(CCH):
                nc.sync.dma_start(xt[:, cc, 1536:2048],
                                  xt_d[cc * 128:(cc + 1) * 128, 1536:2048])
            for cc in range(HL * D // 128):
                nc.sync.dma_start(wprojt[:, cc, :],
                                  wprojt_d[cc * 128:(cc + 1) * 128, :])

            nc.vector.memset(v_aug[:, :, :, D:D + 1], 1.0)

            # ---- phase A tiles: v = x @ w_v^T, one tile per 128-seq chunk.
            # Emitted as fillers (pulled just-in-time by c_unit kc loops).
            def a_tile(nch):
                ps = ps_mm.tile([128, HL * D], F32, tag="mm")
                for cc in range(CCH):
                    nc.tensor.matmul(
                        ps[:],
                        lhsT=xt[:, cc, nch * 128:(nch + 1) * 128],
                        rhs=wvt[:, cc, :],
                        start=(cc == 0),
                        stop=(cc == CCH - 1),
                    )
                    if cc < CCH - 1:
                        yield
                nc.vector.tensor_copy(
                    out=v_aug[:, nch, :, 0:D],
                    in_=ps[:].rearrange("p (h d) -> p h d", h=HL),
                )

            # ---- phase B: qkT = w_qk @ x^T ----
            # One tile = 8-MM accumulation chain + evacuation. Emitted either
            # upfront (what the first C unit needs) or as "filler" work pumped
            # one matmul at a time into the C phase's PE slack.
            def b_tile(mo, qb4, as_gen):
                ps = ps_mm.tile([128, 512], F32, tag="mm")
                for cc in range(CCH):
                    nc.tensor.matmul(
                        ps[:],
                        lhsT=wqkt[:, cc, mo * 128:(mo + 1) * 128],
                        rhs=xt[:, cc, qb4 * 512:(qb4 + 1) * 512],
                        start=(cc == 0),
                        stop=(cc == CCH - 1),
                    )
                    if as_gen and cc < CCH - 1:
                        yield
                nc.vector.tensor_copy(
                    out=qk[:, mo, qb4 * 512:(qb4 + 1) * 512], in_=ps[:])

            # Filler queue, ordered by deadline (just-in-time for C units).
            # CORRECTNESS: Tile's dependency tracking is emission-order based
            # (a read emitted before the producing write becomes a WAR the
            # wrong way and reads garbage silently), so every consumer must
            # ensure() its producer tiles are fully emitted first.
            done_ids = set()
            fillers = []

            # Units run in diagonal (wavefront) order: qb0 is the PE-bound
            # stretch (it builds v and all kT), so spreading pair 1-3's kT
            # builds across later, chain-slack units shortens the critical
            # path. qb3 still completes last (tail proj structure).
            units = [(0, 0), (1, 0), (0, 1), (2, 0), (1, 1), (0, 2),
                     (3, 0), (2, 1), (1, 2), (0, 3), (3, 1), (2, 2),
                     (1, 3), (3, 2), (2, 3), (3, 3)]

            # A tiles + kT/qT B tiles in data-arrival/consumption order (the
            # PE FIFO is strict, so emission order must track DMA arrival).
            # c_units pull what they need via ensure(); the ration pump
            # spreads the slack work evenly.
            fillers.append(("b4t0", b_tile(4, 0, True)))
            fillers.append(("b0t0", b_tile(0, 0, True)))
            for j in range(4):
                for k in range(4 * j, 4 * j + 4):
                    fillers.append((f"a{k}", a_tile(k)))
                if j < 3:
                    fillers.append((f"b4t{j + 1}", b_tile(4, j + 1, True)))
            seen_pairs = {0}
            for p, qb in units[1:]:
                if not (p == 0 and qb == 0):
                    fillers.append((f"b{p}t{qb}", b_tile(p, qb, True)))
                if p not in seen_pairs:
                    seen_pairs.add(p)
                    for j in range(4):
                        fillers.append((f"b{4 + p}t{j}", b_tile(4 + p, j, True)))

            def _step_front():
                # advance the first non-blocked generator (blocked ones are
                # proj tiles waiting on a later pair's normalize; tiles
                # behind them are independent, so skipping is emission-safe)
                for idx in range(len(fillers)):
                    fid, gen = fillers[idx]
                    try:
                        v = next(gen)
                    except StopIteration:
                        done_ids.add(fid)
                        fillers.pop(idx)
                        return 0
                    if v == "blocked":
                        continue
                    return 1
                return -1

            def pump(n):
                emitted = 0
                while emitted < n and fillers:
                    r = _step_front()
                    if r < 0:
                        return
                    emitted += r

            def ensure(fid):
                while fid not in done_ids and fillers:
                    r = _step_front()
                    if r < 0:
                        assert fid in done_ids, f"ensure({fid}) fully blocked"
                        return

            # ration: spread remaining filler matmuls uniformly over the
            # remaining kc iterations (256 total) so the back half of the
            # c phase isn't starved of overlap work.
            TOTAL_KC = 16 * QB * 4
            kc_done = [0]

            def _steps_left():
                n = 0
                for fid, _ in fillers:
                    n += 4 if fid.startswith("proj") else 8
                return n

            def pump_ration():
                kc_done[0] += 1
                left = TOTAL_KC - kc_done[0]
                if left <= 0 or not fillers:
                    return
                need = _steps_left()
                rate = min(3, (need + left - 1) // left)
                pump(rate)

            # ---- phase C: software-pipelined over all 16 (pair, qb) units.
            # The AV matmuls for step j run SKEW steps later, so they never
            # queue the PE behind an exp they'd have to wait for, and the
            # exp chain (the ScalarE metronome) stays gapless across unit
            # boundaries.
            acc_map = {}

            def emit_scores_exp(u, kc):
                p, qb = units[u]
                q0 = qb * QBS
                if kc == 0:
                    ensure(f"b{p}t{qb}")
                ensure(f"b{4 + p}t{kc // 4}")
                sc = ps_sc.tile([128, 1024], F32, tag="sc")
                for par in range(2):     # head 2p (par=0), 2p+1 (par=1)
                    pp = par * 64
                    nc.tensor.matmul(
                        sc[:, par * 512:(par + 1) * 512],
                        lhsT=qk[pp:pp + 64, 4 + p, kc * 128:(kc + 1) * 128],
                        rhs=qk[pp:pp + 64, p, q0:q0 + QBS],
                        start=True,
                        stop=True,
                    )
                e_t = e_pool.tile([128, 1024], BF16, tag="e")
                nc.scalar.activation(
                    out=e_t[:], in_=sc[:],
                    func=mybir.ActivationFunctionType.Exp, scale=SCALE)
                return e_t

            def emit_av(u, kc, e_t):
                p, qb = units[u]
                ensure(f"a{kc}")
                if kc == 0:
                    acc = ps_out.tile([65, 1024], F32, tag="acc", name="acc")
                    acc_map[u] = acc
                acc = acc_map[u]
                nc.tensor.matmul(
                    acc[:, 0:512], lhsT=v_aug[:, kc, 2 * p, 0:D + 1],
                    rhs=e_t[:, 0:512],
                    start=(kc == 0), stop=(kc == KC - 1))
                nc.tensor.matmul(
                    acc[:, 512:1024], lhsT=v_aug[:, kc, 2 * p + 1, 0:D + 1],
                    rhs=e_t[:, 512:1024],
                    start=(kc == 0), stop=(kc == KC - 1))

            def finish_unit(u):
                # evacuate numerator + sumexp to release the PSUM accumulator;
                # then recip on DVE + GpSimd partition broadcast (engine-local,
                # no DRAM round-trip) and the normalize multiply.
                p, qb = units[u]
                q0 = qb * QBS
                acc = acc_map.pop(u)
                last = (u == len(units) - 1)
                nus, ses = [], []
                for par in range(2):
                    c0 = par * 512
                    srow = tiny.tile([1, QBS], F32, tag="srow")
                    nu_t = tiny.tile([64, QBS], BF16, tag="nu")
                    if last and par == 1:
                        # ScalarE is idle once the exp chain ends; split the
                        # final accumulator evacuation across both engines.
                        nc.scalar.copy(out=srow[:], in_=acc[64:65, c0:c0 + 512])
                        nc.scalar.copy(out=nu_t[:], in_=acc[0:64, c0:c0 + 512])
                    else:
                        nc.vector.tensor_copy(
                            out=srow[:], in_=acc[64:65, c0:c0 + 512])
                        nc.vector.tensor_copy(
                            out=nu_t[:], in_=acc[0:64, c0:c0 + 512])
                    nus.append(nu_t)
                    ses.append(srow)
                # reciprocal must run packed ([128, 4]: DVE recip is ~8
                # cyc/elem, so a wide or single-row call costs us); pack and
                # unpack via SBUF->SBUF DMA (overlaps DVE work), broadcast
                # across partitions on the otherwise-idle GpSimd.
                for par in range(2):
                    g = tiny.tile([128, QBS // 128], F32, tag="g")
                    nc.sync.dma_start(g[:], ses[par][:])
                    r = tiny.tile([128, QBS // 128], F32, tag="r")
                    nc.vector.reciprocal(out=r[:], in_=g[:])
                    rrow = tiny.tile([1, QBS], F32, tag="rrow")
                    nc.sync.dma_start(rrow[:], r[:])
                    bc = tiny.tile([64, QBS], F32, tag="bc")
                    nc.gpsimd.partition_broadcast(bc[:], rrow[:], channels=64)
                    pp = par * 64
                    nc.vector.tensor_mul(
                        outT[pp:pp + 64, p, q0:q0 + QBS], nus[par][:], bc[:])
                normed.add((p, qb))
                if p == 3 and qb < 3:
                    for j in range(4):
                        for ob in range(2):
                            fillers.append(
                                (f"proj{qb}_{j}_{ob}", proj_tile(qb * 4 + j, ob)))
                if p == 1 and qb == 3:
                    # front-insert: the stage-1 prefill (pairs 0-1) must beat
                    # qb2's proj tiles to the pump so it completes before the
                    # final norm; all a/b producers are done by this point.
                    for j in range(4):
                        for ob in range(2):
                            fillers.insert(
                                2 * j + ob,
                                (f"proj3_{j}_{ob}", proj_tail_tile(12 + j, ob)))

            normed = set()

            def proj_tile(nch, ob):
                ps = ps_mm.tile([128, 512], F32, tag="mm")
                ncc = HL * D // 128
                for cc in range(ncc):
                    nc.tensor.matmul(
                        ps[:],
                        lhsT=outT[:, cc, nch * 128:(nch + 1) * 128],
                        rhs=wprojt[:, cc, ob * 512:(ob + 1) * 512],
                        start=(cc == 0),
                        stop=(cc == ncc - 1),
                    )
                    if cc < ncc - 1:
                        yield
                st = stage.tile([128, 512], F32, tag="st")
                nc.vector.tensor_copy(out=st[:], in_=ps[:])
                nc.sync.dma_start(
                    out_d[nch * 128:(nch + 1) * 128,
                          ob * 512:(ob + 1) * 512], st[:])

            def proj_tail_tile(nch, ob):
                # last q-block only: two-stage so pairs 0-1 are contracted
                # (and the PSUM buf released) while units 14/15 still run;
                # after the final norm only cc2+cc3 and a DVE add remain.
                ps = ps_mm.tile([128, 512], F32, tag="mm")
                for cc in range(2):
                    nc.tensor.matmul(
                        ps[:],
                        lhsT=outT[:, cc, nch * 128:(nch + 1) * 128],
                        rhs=wprojt[:, cc, ob * 512:(ob + 1) * 512],
                        start=(cc == 0), stop=(cc == 1))
                    yield
                part = ppool.tile([128, 512], F32, tag="pp")
                nc.vector.tensor_copy(out=part[:], in_=ps[:])
                yield
                while (3, 3) not in normed:
                    yield "blocked"
                ps2 = ps_mm.tile([128, 512], F32, tag="mm")
                for cc in range(2, 4):
                    nc.tensor.matmul(
                        ps2[:],
                        lhsT=outT[:, cc, nch * 128:(nch + 1) * 128],
                        rhs=wprojt[:, cc, ob * 512:(ob + 1) * 512],
                        start=(cc == 2), stop=(cc == 3))
                    yield
                st = stage.tile([128, 512], F32, tag="st")
                nc.vector.tensor_add(out=st[:], in0=part[:], in1=ps2[:])
                nc.sync.dma_start(
                    out_d[nch * 128:(nch + 1) * 128,
                          ob * 512:(ob + 1) * 512], st[:])

            # AV lags scores/exp by SKEW steps so (a) AV never makes the PE
            # FIFO wait on an in-flight exp and (b) the single-buffered acc's
            # evacuation (DVE) has 2 steps of slack before the next unit's
            # first AV needs the banks back.
            SKEW = 5
            av_q = []
            def _drain_one():
                pu, pkc, pe = av_q.pop(0)
                emit_av(pu, pkc, pe)
                if pkc == KC - 1:
                    finish_unit(pu)
            for j in range(KC * len(units)):
                u, kc = divmod(j, KC)
                e_t = emit_scores_exp(u, kc)
                av_q.append((u, kc, e_t))
                while len(av_q) > SKEW:
                    _drain_one()
                pump_ration()
            while av_q:
                _drain_one()
            while fillers:
                pump(1000)

            if dumps:
                nc.sync.dma_start(qk_dump[:], qk[:])
                nc.sync.dma_start(vaug_dump[:], v_aug[:])
                nc.sync.dma_start(outT_dump[:], outT[:])

    nc.compile()
    _CACHE["nc"] = nc
    return nc


# ---------------------------------------------------------------------------
# host wrapper
# ---------------------------------------------------------------------------
def kernel(x, w_qkv, w_proj, b_proj):
    _install_ntff_hook()
    from concourse.bass_utils import run_bass_kernel_spmd

    x = np.asarray(x, dtype=np.float32)
    w_qkv = np.asarray(w_qkv, dtype=np.float32)
    w_proj = np.asarray(w_proj, dtype=np.float32)
    b_proj = np.asarray(b_proj, dtype=np.float32)
    B = x.shape[0]

    nc = _build()

    def bf(a):
        return np.ascontiguousarray(a).astype(ml_dtypes.bfloat16)

    in_maps = []
    for core in range(NCORES):
        b, hg = core // 2, core % 2
        sl = slice(hg * HL * D, (hg + 1) * HL * D)
        w_q = w_qkv[0 * C:1 * C][sl]            # [512, C]
        w_k = w_qkv[1 * C:2 * C][sl]
        w_v = w_qkv[2 * C:3 * C][sl]
        w_qk_t = np.concatenate([w_q, w_k], axis=0).T   # [C, 1024]
        in_maps.append({
            "x_t": bf(x[b].T),                  # [C, N]
            "w_qkt": bf(w_qk_t),
            "w_vt": bf(w_v.T),                  # [C, 512]
            "w_projt": bf(w_proj[:, sl].T),     # [512, C]
        })

    trace = bool(int(os.environ.get("KERNEL_TRACE", "0")))
    res = run_bass_kernel_spmd(nc, in_maps, core_ids=list(range(NCORES)),
                               trace=trace)
    kernel.last_results = res

    out = np.empty((B, N, C), dtype=np.float32)
    for b in range(B):
        out[b] = res.results[2 * b]["out"] + res.results[2 * b + 1]["out"]
        out[b] += b_proj
    return out


if __name__ == "__main__":
    t0 = time.time()
    _build()
    print(f"build+compile: {time.time()-t0:.1f}s")

